# revision 16
# baseline (speedup 1.0000x reference)
"""Linformer attention block on 8 TRN2 NeuronCores, data-parallel over batch.

v3: fp8e4 DoubleRow (0.5 cyc/col) for projection/attention matmuls where
walrus allows it (128-out-partition at offset 0); plain fp8 elsewhere (dots,
attn@v second head). K/V projections reassociated as (proj^T y) @ W. bf16
ones-matmul LN stats (fp32r rejected by the BIR verifier), stats staging on
the Pool engine. Two-phase emission per rep (all LN/projections, then all
attention) so ACT loads each activation table once. Softmax denominators via
zero-padded fp8 DoubleRow ones-matmul into psum rows 0:2. x+pos via resident
pos tile + Pool add (HBM read once). Wo bias folded into pos host-side.

Scale bookkeeping (host pre-scales, device compensates at evictions):
  y8 = 16 y;  wq8 = 512 (Wq dh^-.5);  q8 = 128 q;   pk8/pv8 = 256 proj
  yk8 = 16 yk; wk8/wv8/wo8 = 64 W;    k8 = 32 k;    v8 = 32 v;  ao8 = 32 ao
  dots_psum = 4096 dots  -> exp scale 1/4096
  wo_psum  = 2048 out    -> final evict scale 1/2048
"""

import os
import sys
import types

import numpy as np
import ml_dtypes

try:
    import antenv.axon_hooks  # noqa: F401
except ImportError:
    _shim = types.ModuleType("antenv.axon_hooks")
    _shim.get_axon_ntff_profile_hook = lambda: None
    sys.modules["antenv.axon_hooks"] = _shim

import concourse.bass as bass
import concourse.mybir as mybir
from concourse import bacc
from concourse.tile import TileContext
from concourse.bass_utils import run_bass_kernel_spmd

F32 = mybir.dt.float32
BF16 = mybir.dt.bfloat16
F8 = mybir.dt.float8e4
OP = mybir.AluOpType
AF = mybir.ActivationFunctionType
DR = mybir.MatmulPerfMode.DoubleRow

B, C, HH, WW = 32, 512, 32, 32
N = HH * WW            # 1024
HEADS = 8
DH = C // HEADS        # 64
KLR = 256              # linformer rank
EPS = 1e-5
NCORES = 8
BL = B // NCORES       # 4 batch elems per core
CC = C // 128          # 4 channel chunks
NH = N // 512          # 2 free halves
KC = KLR // 128        # 2 k chunks
NT = N // 128          # 8 token chunks

S_Y, S_WQ, S_Q = 16.0, 512.0, 128.0
S_P, S_YK, S_W, S_K, S_V = 256.0, 16.0, 64.0, 32.0, 32.0


def _rearr(d):
    return d[:].rearrange("(a p) n -> p a n", p=128)


def _build(reps=1):
    nc = bacc.Bacc()
    dp = nc.declare_dram_parameter
    x_d = dp("x", [BL, C, N], F32, isOutput=False)
    posT_d = dp("posT", [C, N], F32, isOutput=False)
    wq_d = dp("wq", [C, C], F8, isOutput=False)
    wk_d = dp("wk", [C, C], F8, isOutput=False)
    wv_d = dp("wv", [C, C], F8, isOutput=False)
    wo_d = dp("wo", [C, C], F8, isOutput=False)
    pk_d = dp("pk", [N, KLR], F8, isOutput=False)
    pv_d = dp("pv", [N, KLR], F8, isOutput=False)
    ident_d = dp("ident", [128, 128], F8, isOutput=False)
    sel8_d = dp("sel8", [8, CC, 128], BF16, isOutput=False)
    e2b_d = dp("e2b", [128, CC, 2, 2, 128], F8, isOutput=False)
    gcol_d = dp("gcol", [128, CC], F32, isOutput=False)
    lnb16_d = dp("lnb16", [128, CC], F32, isOutput=False)
    out_d = dp("out", [BL, C, N], F32, isOutput=True)

    with TileContext(nc) as tc:
        with (
            tc.tile_pool(name="const", bufs=1) as cp,
            tc.tile_pool(name="work", bufs=2) as wp,
        ):
            posT = cp.tile([128, CC, N], F32)
            nc.sync.dma_start(out=posT, in_=_rearr(posT_d))
            wq = cp.tile([128, CC, C], F8)
            nc.sync.dma_start(out=wq, in_=_rearr(wq_d))
            wk = cp.tile([128, CC, C], F8)
            nc.sync.dma_start(out=wk, in_=_rearr(wk_d))
            wv = cp.tile([128, CC, C], F8)
            nc.sync.dma_start(out=wv, in_=_rearr(wv_d))
            wo = cp.tile([128, CC, C], F8)
            nc.sync.dma_start(out=wo, in_=_rearr(wo_d))
            pk = cp.tile([128, NT, KLR], F8)
            nc.sync.dma_start(out=pk, in_=_rearr(pk_d))
            pv = cp.tile([128, NT, KLR], F8)
            nc.sync.dma_start(out=pv, in_=_rearr(pv_d))
            ident = cp.tile([128, 128], F8)
            nc.sync.dma_start(out=ident, in_=ident_d[:])
            gcol = cp.tile([128, CC], F32)
            nc.sync.dma_start(out=gcol, in_=gcol_d[:])
            lnb16 = cp.tile([128, CC], F32)
            nc.sync.dma_start(out=lnb16, in_=lnb16_d[:])

            onesb = cp.tile([128, 1], BF16)
            nc.vector.memset(onesb, 1.0)
            onesrow = cp.tile([1, 128], BF16)
            nc.vector.memset(onesrow, 1.0)
            sel8 = cp.tile([8, CC, 128], BF16)
            nc.sync.dma_start(out=sel8, in_=sel8_d[:])
            # sums lhsT: e2b[:, pr, hp, kc, j] = 1 iff j == 2*pr+hp
            # (zero-padded to 128 cols; DoubleRow needs full-width weights)
            e2b = cp.tile([128, CC, 2, 2, 128], F8)
            nc.sync.dma_start(out=e2b, in_=e2b_d[:])
            epsc = cp.tile([1, 1], F32)
            nc.vector.memset(epsc, EPS)

            c = dict(wq=wq, wk=wk, wv=wv, wo=wo, pk=pk, pv=pv, ident=ident,
                     gcol=gcol, lnb16=lnb16, onesb=onesb, onesrow=onesrow,
                     sel8=sel8, e2b=e2b, epsc=epsc, posT=posT)
            with nc.allow_low_precision(reason="fp8 attention path"):
                for _rep in range(reps):
                    fronts = []
                    with (tc.tile_pool(name=f"psA{_rep}", bufs=2,
                                       space="PSUM") as ppa,
                          tc.tile_pool(name=f"wA{_rep}", bufs=2) as wpa):
                        for b0 in range(0, BL, 2):
                            fronts.extend(
                                _emit_front_pair(nc, wp, wpa, ppa,
                                                 (b0, b0 + 1), x_d, c))
                    with (tc.tile_pool(name=f"psB{_rep}", bufs=2,
                                       space="PSUM") as ppb,
                          tc.tile_pool(name=f"wB{_rep}", bufs=2) as wpb):
                        _emit_back_all(nc, wp, wpb, ppb, out_d, c, fronts)
    nc.compile()
    return nc


def _emit_front_pair(nc, wp, wpa, pp, pair, x_d, c):
    """LN + q/k/v projections for a pair of batches, stage-interleaved.
    ACT funcs: Sqrt, Relu, Copy."""
    wq, wk, wv = c["wq"], c["wk"], c["wv"]
    pk, pv = c["pk"], c["pv"]
    gcol, lnb16, epsc = c["gcol"], c["lnb16"], c["epsc"]
    onesb, onesrow, posT = c["onesb"], c["onesrow"], c["posT"]
    F = {b: {} for b in pair}

    # ------- s = x + posT' in bf16 (posT' carries Wo bias) -------
    for b in pair:
        x = wpa.tile([128, CC, N], F32, tag="x", bufs=2)
        nc.sync.dma_start(out=x,
                          in_=x_d[b].rearrange("(a p) n -> p a n", p=128))
        s = wp.tile([128, CC, N], BF16, tag="s", bufs=BL,
                    name=f"s_{b}")
        nc.vector.tensor_add(s, x, posT)
        F[b]["s"] = s

    # ------- LN statistics (sum via s directly; sumsq via pool squares) ----
    for b in pair:
        s = F[b]["s"]
        minis = wpa.tile([1, 2, N], BF16, tag="minis", bufs=2,
                         name=f"minis_{b}")
        m1c = wpa.tile([1, 2, 512], F32, tag="m1c", bufs=2,
                       name=f"m1c_{b}")  # C*mean^2 per half
        for nh in range(NH):
            nsl = slice(nh * 512, (nh + 1) * 512)
            s1 = pp.tile([1, 512], F32, tag="st", bufs=2)
            for cc in range(CC):
                nc.tensor.matmul(s1, onesb[:], s[:, cc, nsl],
                                 start=(cc == 0), stop=(cc == CC - 1))
            mean = minis[0:1, 0, nsl]
            nc.vector.tensor_scalar_mul(mean, s1, 1.0 / C)
            nc.vector.tensor_mul(m1c[0:1, nh, :], mean, s1)  # = C*mean^2
        F[b]["minis"], F[b]["m1c"] = minis, m1c
    for b in pair:
        s = F[b]["s"]
        s2s = [pp.tile([1, 512], F32, tag="st", bufs=2, name=f"s2_{b}_{i}")
               for i in range(NH)]
        for cc in range(CC):
            sq = wpa.tile([128, N], BF16, tag="sq", bufs=4)
            nc.vector.tensor_mul(sq, s[:, cc, :], s[:, cc, :])
            for nh in range(NH):
                nsl = slice(nh * 512, (nh + 1) * 512)
                nc.tensor.matmul(s2s[nh], onesb[:], sq[:, nsl],
                                 start=(cc == 0), stop=(cc == CC - 1))
        F[b]["s2s"] = s2s
    for b in pair:
        minis, m1c = F[b]["minis"], F[b]["m1c"]
        for nh in range(NH):
            nsl = slice(nh * 512, (nh + 1) * 512)
            s2 = F[b]["s2s"][nh]
            v512 = wpa.tile([1, 512], F32, tag="mini2", bufs=4)
            nc.vector.scalar_tensor_tensor(v512, in0=m1c[0:1, nh, :],
                                           scalar=-1.0, in1=s2,
                                           op0=OP.mult, op1=OP.add)  # C*var
            sd = wpa.tile([1, 512], F32, tag="mini2", bufs=4)
            nc.scalar.activation(sd, v512, AF.Sqrt, bias=epsc[:],
                                 scale=1.0 / C)
            nc.vector.reciprocal(minis[0:1, 1, nsl], sd)

    # broadcast mean/rstd down partitions; evict to bf16 SBUF
    for b in pair:
        bc = pp.tile([128, 2, N], F32, tag="bc", bufs=1)
        for j in range(2):
            for nh in range(NH):
                nsl = slice(nh * 512, (nh + 1) * 512)
                nc.tensor.matmul(bc[:, j, nsl], onesrow[:],
                                 F[b]["minis"][0:1, j, nsl],
                                 start=True, stop=True)
        bcs = wpa.tile([128, 2, N], BF16, tag="bcs", bufs=2)
        nc.scalar.copy(bcs, bc)
        F[b]["bcs"] = bcs

    # ------- LN apply + relu -> y8 (fp8, x16) -------
    for b in pair:
        F[b]["y8"] = wpa.tile([128, CC, N], F8, tag="y8", bufs=2,
                                  name=f"y8_{b}")
    for cc in range(CC):
        for b in pair:
            s, bcs, y8 = F[b]["s"], F[b]["bcs"], F[b]["y8"]
            t1 = wpa.tile([128, N], BF16, tag="lnt", bufs=4)
            nc.vector.tensor_sub(t1, s[:, cc, :], bcs[:, 0, :])
            t2 = wpa.tile([128, N], BF16, tag="lnt", bufs=4)
            nc.vector.scalar_tensor_tensor(t2, in0=t1,
                                           scalar=gcol[:, cc:cc + 1],
                                           in1=bcs[:, 1, :], op0=OP.mult,
                                           op1=OP.mult)
            nc.scalar.activation(y8[:, cc, :], t2, AF.Relu,
                                 bias=lnb16[:, cc:cc + 1], scale=S_Y)

    # ------- qT [d_part, dc, n] fp8 (x128) -------
    for b in pair:
        F[b]["q8"] = wp.tile([128, CC, N], F8, tag="q8", bufs=BL,
                                 name=f"q8_{b}")
    for dc in range(CC):
        for b in pair:
            y8, q8 = F[b]["y8"], F[b]["q8"]
            for nh in range(NH):
                nsl = slice(nh * 512, (nh + 1) * 512)
                ps = pp.tile([128, 512], F32, tag="pj", bufs=2)
                for kcp in range(0, CC, 2):
                    nc.tensor.matmul(
                        ps, wq[:, kcp:kcp + 2, dc * 128:(dc + 1) * 128],
                        y8[:, kcp:kcp + 2, nsl], perf_mode=DR,
                        start=(kcp == 0), stop=(kcp == CC - 2))
                if nh == 0:
                    nc.vector.tensor_scalar_mul(q8[:, dc, nsl], ps,
                                                S_Q / (S_Y * S_WQ))
                else:
                    nc.scalar.activation(q8[:, dc, nsl], ps, AF.Copy,
                                         scale=S_Q / (S_Y * S_WQ))

    # ------- kbf/vbf [n_part, t, d] fp8: transpose fused into the matmul
    # (y8 as the stationary operand), DoubleRow over channel pairs -------
    for which, w8 in (("k", wk), ("v", wv)):
        for b in pair:
            F[b][f"{which}bf"] = wpa.tile([128, NT, C], F8,
                                          tag=f"{which}bf", bufs=2,
                                          name=f"{which}bf_{b}")
        for t in range(NT):
            for b in pair:
                y8 = F[b]["y8"]
                ps = pp.tile([128, 512], F32, tag="pj", bufs=2)
                for kcp in range(0, CC, 2):
                    nc.tensor.matmul(
                        ps, y8[:, kcp:kcp + 2, t * 128:(t + 1) * 128],
                        w8[:, kcp:kcp + 2, :], perf_mode=DR,
                        start=(kcp == 0), stop=(kcp == CC - 2))
                dst = F[b][f"{which}bf"][:, t, :]
                if t % 2 == 0:
                    nc.vector.tensor_scalar_mul(dst, ps, 1.0 / S_W)
                else:
                    nc.scalar.activation(dst, ps, AF.Copy, scale=1.0 / S_W)

    # ------- ktbf [d_part, dc, K] = kbf^T proj_k (x S_K) -------
    for b in pair:
        ktbf = wp.tile([128, CC, KLR], F8, tag="ktbf", bufs=BL,
                       name=f"ktbf_{b}")
        F[b]["ktbf"] = ktbf
    for dcp in range(0, CC, 2):
        for b in pair:
            kbf = F[b]["kbf"]
            ps = pp.tile([128, 512], F32, tag="pj", bufs=2)
            for i in range(2):
                dc = dcp + i
                for tp in range(0, NT, 2):
                    nc.tensor.matmul(
                        ps[:, i * 256:(i + 1) * 256],
                        kbf[:, tp:tp + 2, dc * 128:(dc + 1) * 128],
                        pk[:, tp:tp + 2, :], perf_mode=DR,
                        start=(tp == 0), stop=(tp == NT - 2),
                        skip_group_check=True)
            nc.scalar.activation(F[b]["ktbf"][:, dcp:dcp + 2, :], ps, AF.Copy,
                                 scale=S_K / (S_Y * S_P))

    # ------- vsbf [k_part, kc, d] = proj_v^T vbf (x S_V) -------
    for b in pair:
        F[b]["vsbf"] = wp.tile([128, KC, C], F8, tag="vsbf", bufs=BL,
                                   name=f"vsbf_{b}")
    for kc in range(KC):
        for b in pair:
            vbf = F[b]["vbf"]
            ps = pp.tile([128, 512], F32, tag="pj", bufs=2)
            for tp in range(0, NT, 2):
                nc.tensor.matmul(ps, pv[:, tp:tp + 2, kc * 128:(kc + 1) * 128],
                                 vbf[:, tp:tp + 2, :], perf_mode=DR,
                                 start=(tp == 0), stop=(tp == NT - 2))
            nc.vector.tensor_scalar_mul(F[b]["vsbf"][:, kc, :], ps,
                                        S_V / (S_Y * S_P))

    return [dict(s=F[b]["s"], q8=F[b]["q8"], ktbf=F[b]["ktbf"],
                 vsbf=F[b]["vsbf"]) for b in pair]


def _emit_back_all(nc, wp, wpb, pp, out_d, c, fronts):
    """Attention + Wo + residual, 3-stage skewed pipeline across the 64
    (nh, pr, batch) steps so PE never queues a matmul right behind the
    ACT/DVE round-trip it depends on. ACT funcs: Exp, Copy."""
    wo, sel2, e2 = c["wo"], c["sel2"], c["e2"]

    for f in fronts:
        f["aobf"] = wpb.tile([128, CC, N], F8, tag="aobf", bufs=BL,
                             name=f"aobf_{id(f)}")
    steps = [(nh, pr, b) for nh in range(NH) for pr in range(CC)
             for b in range(len(fronts))]
    state = {}

    def stage_a(i):
        nh, pr, b = steps[i]
        nsl = slice(nh * 512, (nh + 1) * 512)
        f = fronts[b]
        attn = {}
        for hp in range(2):
            rsl = slice(hp * 64, (hp + 1) * 64)
            dps = pp.tile([128, 2, 512], F32, tag="dp", bufs=2,
                          name=f"dps_{i}_{hp}")
            for kc in range(KC):
                nc.tensor.matmul(dps[:, kc, :],
                                 f["ktbf"][rsl, pr, kc * 128:(kc + 1) * 128],
                                 f["q8"][rsl, pr, nsl],
                                 start=True, stop=True)
            at = wpb.tile([128, 2, 512], F8, tag="attn", bufs=12,
                          name=f"at_{i}_{hp}")
            attn[hp] = at
            nc.scalar.activation(at, dps, AF.Exp, scale=1.0 / (S_Q * S_K))
        state[i] = dict(attn=attn)

    def stage_b(i):
        nh, pr, b = steps[i]
        st = state[i]
        ms = pp.tile([128, 2, 512], F32, tag="ms", bufs=2, name=f"ms_{i}")
        for hp in range(2):
            nc.tensor.matmul(ms[:, 0, :], e2[:, hp, :, :],
                             st["attn"][hp][:, :, :], perf_mode=DR,
                             start=(hp == 0), stop=(hp == 1),
                             skip_group_check=True)
        recip = wpb.tile([2, 512], BF16, tag="recip", bufs=4,
                         name=f"recip_{i}")
        nc.vector.reciprocal(recip, ms[0:2, 0, :])
        st["ms"], st["recip"] = ms, recip

    def stage_c(i):
        nh, pr, b = steps[i]
        nsl = slice(nh * 512, (nh + 1) * 512)
        f = fronts[b]
        st = state.pop(i)
        ms, attn = st["ms"], st["attn"]
        nc.tensor.matmul(ms[:, 1, :], sel2[:], st["recip"][:],
                         start=True, stop=True, skip_group_check=True)
        rbc = wpb.tile([128, 512], BF16, tag="rbc", bufs=4,
                       name=f"rbcs_{i}")
        nc.scalar.copy(rbc, ms[:, 1, :])
        # attn@v reuses the sums bank (freed by the reciprocal): hp0 via
        # DoubleRow at rows 0:64; DR cannot write at offset 64, so hp1
        # uses plain fp8 there
        av = ms[:, 0, :]
        vsbf = f["vsbf"]
        nc.tensor.matmul(av[0:64, :],
                         vsbf[:, :, (2 * pr) * 64:(2 * pr + 1) * 64],
                         attn[0][:, :, :], perf_mode=DR,
                         start=True, stop=True, skip_group_check=True)
        h1 = 2 * pr + 1
        for kc in range(KC):
            nc.tensor.matmul(av[64:128, :],
                             vsbf[:, kc, h1 * 64:(h1 + 1) * 64],
                             attn[1][:, kc, :],
                             start=(kc == 0), stop=(kc == KC - 1),
                             tile_position=(0, 64), skip_group_check=True)
        nc.vector.scalar_tensor_tensor(f["aobf"][:, pr, nsl], in0=av,
                                       scalar=0.0, in1=rbc,
                                       op0=OP.bypass, op1=OP.mult)

    n = len(steps)
    for i in range(n + 2):
        if i < n:
            stage_a(i)
        if 1 <= i and i - 1 < n and i >= 2:
            pass
        if i >= 1 and i - 1 < n:
            stage_b(i - 1)
        if i >= 2:
            stage_c(i - 2)

    # ------- Wo + residual (+bias via posT') -> out -------
    for co in range(CC):
        for b, f in enumerate(fronts):
            outf = wpb.tile([128, N], F32, tag="outf", bufs=4)
            for nh in range(NH):
                nsl = slice(nh * 512, (nh + 1) * 512)
                ps = pp.tile([128, 2, 512], F32, tag="dp", bufs=2,
                             name=f"wo_{b}_{co}_{nh}")
                for pp_ in range(0, CC, 2):
                    nc.tensor.matmul(ps[:, 0, :],
                                     wo[:, pp_:pp_ + 2,
                                        co * 128:(co + 1) * 128],
                                     f["aobf"][:, pp_:pp_ + 2, nsl],
                                     perf_mode=DR,
                                     start=(pp_ == 0), stop=(pp_ == CC - 2))
                nc.vector.scalar_tensor_tensor(outf[:, nsl], in0=ps[:, 0, :],
                                               scalar=1.0 / (S_V * S_W),
                                               in1=f["s"][:, co, nsl],
                                               op0=OP.mult, op1=OP.add)
            nc.sync.dma_start(out=out_d[b, co * 128:(co + 1) * 128, :],
                              in_=outf)


_CACHE = {}


def get_nc(reps=1):
    key = ("nc", reps)
    if key not in _CACHE:
        _CACHE[key] = _build(reps)
    return _CACHE[key]


def _sel8_host():
    sel8 = np.zeros((8, CC, 128), ml_dtypes.bfloat16)
    for pr in range(CC):
        sel8[2 * pr, pr, 0:64] = 1
        sel8[2 * pr + 1, pr, 64:128] = 1
    return sel8


def _e2b_host():
    e2b = np.zeros((128, CC, 2, 2, 128), ml_dtypes.float8_e4m3)
    for pr in range(CC):
        for hp in range(2):
            e2b[:, pr, hp, :, 2 * pr + hp] = 1
    return e2b


def make_in_maps(inputs):
    f8 = ml_dtypes.float8_e4m3
    x = np.ascontiguousarray(np.asarray(inputs["x"], np.float32)
                             .reshape(B, C, N))
    pos = np.asarray(inputs["pos"], np.float32).reshape(N, C)
    ln_g = np.asarray(inputs["ln_g"], np.float32)
    ln_b = np.asarray(inputs["ln_b"], np.float32)
    bo = np.asarray(inputs["bo"], np.float32)

    posT = np.ascontiguousarray(pos.T) + bo[:, None]  # fold Wo bias into pos

    shared = {
        "posT": posT,
        "wq": (np.asarray(inputs["Wq"], np.float32)
               * (DH ** -0.5) * S_WQ).astype(f8),
        "wk": (np.asarray(inputs["Wk"], np.float32) * S_W).astype(f8),
        "wv": (np.asarray(inputs["Wv"], np.float32) * S_W).astype(f8),
        "wo": (np.asarray(inputs["Wo"], np.float32) * S_W).astype(f8),
        "pk": (np.asarray(inputs["proj_k"], np.float32) * S_P).astype(f8),
        "pv": (np.asarray(inputs["proj_v"], np.float32) * S_P).astype(f8),
        "ident": np.eye(128, dtype=f8),
        "sel8": _sel8_host(),
        "e2b": _e2b_host(),
        "gcol": np.ascontiguousarray(ln_g.reshape(CC, 128).T),
        "lnb16": np.ascontiguousarray((S_Y * ln_b).reshape(CC, 128).T),
    }
    return [dict(shared, x=np.ascontiguousarray(x[i * BL:(i + 1) * BL]))
            for i in range(NCORES)]


def kernel(**inputs):
    nc = get_nc()
    in_maps = make_in_maps(inputs)
    trace = bool(int(os.environ.get("BASS_KERNEL_TRACE", "0")))
    res = run_bass_kernel_spmd(nc, in_maps, core_ids=list(range(NCORES)),
                               trace=trace)
    kernel.last_result = res
    out = np.concatenate([np.asarray(res.results[i]["out"], np.float32)
                          [None] for i in range(NCORES)], axis=0)
    return np.ascontiguousarray(out.reshape(B, C, HH, WW))


# revision 20
# speedup vs baseline: 1.3021x; 1.3021x over previous
"""Linformer attention block on 8 TRN2 NeuronCores, data-parallel over batch.

v3: fp8e4 DoubleRow (0.5 cyc/col) for projection/attention matmuls where
walrus allows it (128-out-partition at offset 0); plain fp8 elsewhere (dots,
attn@v second head). K/V projections reassociated as (proj^T y) @ W. bf16
ones-matmul LN stats (fp32r rejected by the BIR verifier), stats staging on
the Pool engine. Two-phase emission per rep (all LN/projections, then all
attention) so ACT loads each activation table once. Softmax denominators via
zero-padded fp8 DoubleRow ones-matmul into psum rows 0:2. x+pos via resident
pos tile + Pool add (HBM read once). Wo bias folded into pos host-side.

Scale bookkeeping (host pre-scales, device compensates at evictions):
  y8 = 16 y;  wq8 = 512 (Wq dh^-.5);  q8 = 128 q;   pk8/pv8 = 256 proj
  yk8 = 16 yk; wk8/wv8/wo8 = 64 W;    k8 = 32 k;    v8 = 32 v;  ao8 = 32 ao
  dots_psum = 4096 dots  -> exp scale 1/4096
  wo_psum  = 2048 out    -> final evict scale 1/2048
"""

import os
import sys
import types

import numpy as np
import ml_dtypes

try:
    import antenv.axon_hooks  # noqa: F401
except ImportError:
    _shim = types.ModuleType("antenv.axon_hooks")
    _shim.get_axon_ntff_profile_hook = lambda: None
    sys.modules["antenv.axon_hooks"] = _shim

import concourse.bass as bass
import concourse.mybir as mybir
from concourse import bacc
from concourse.tile import TileContext
from concourse.bass_utils import run_bass_kernel_spmd

F32 = mybir.dt.float32
BF16 = mybir.dt.bfloat16
F8 = mybir.dt.float8e4
OP = mybir.AluOpType
AF = mybir.ActivationFunctionType
DR = (None if os.environ.get("BASS_NO_DR") else
      mybir.MatmulPerfMode.DoubleRow)

B, C, HH, WW = 32, 512, 32, 32
N = HH * WW            # 1024
HEADS = 8
DH = C // HEADS        # 64
KLR = 256              # linformer rank
EPS = 1e-5
NCORES = 8
BL = B // NCORES       # 4 batch elems per core
CC = C // 128          # 4 channel chunks
NH = N // 512          # 2 free halves
KC = KLR // 128        # 2 k chunks
NT = N // 128          # 8 token chunks

S_Y, S_WQ, S_Q = 16.0, 512.0, 128.0
S_P, S_YK, S_W, S_K, S_V = 256.0, 16.0, 64.0, 32.0, 32.0


def _rearr(d):
    return d[:].rearrange("(a p) n -> p a n", p=128)


def _build(reps=1):
    nc = bacc.Bacc()
    dp = nc.declare_dram_parameter
    x_d = dp("x", [BL, C, N], F32, isOutput=False)
    posT_d = dp("posT", [C, N], F32, isOutput=False)
    wq_d = dp("wq", [C, C], F8, isOutput=False)
    wk_d = dp("wk", [C, C], F8, isOutput=False)
    wv_d = dp("wv", [C, C], F8, isOutput=False)
    wo_d = dp("wo", [C, C], F8, isOutput=False)
    pk_d = dp("pk", [N, KLR], F8, isOutput=False)
    pv_d = dp("pv", [N, KLR], F8, isOutput=False)
    ident_d = dp("ident", [128, 128], F8, isOutput=False)
    sel8_d = dp("sel8", [8, CC, 128], BF16, isOutput=False)
    e2b_d = dp("e2b", [128, CC, 2, 2, 128], F8, isOutput=False)
    gcol_d = dp("gcol", [128, CC], F32, isOutput=False)
    lnb16_d = dp("lnb16", [128, CC], F32, isOutput=False)
    out_d = dp("out", [BL, C, N], F32, isOutput=True)

    with TileContext(nc) as tc:
        with (
            tc.tile_pool(name="const", bufs=1) as cp,
            tc.tile_pool(name="work", bufs=2) as wp,
        ):
            posT = cp.tile([128, CC, N], F32)
            nc.sync.dma_start(out=posT, in_=_rearr(posT_d))
            wq = cp.tile([128, CC, C], F8)
            nc.sync.dma_start(out=wq, in_=_rearr(wq_d))
            wk = cp.tile([128, CC, C], F8)
            nc.sync.dma_start(out=wk, in_=_rearr(wk_d))
            wv = cp.tile([128, CC, C], F8)
            nc.sync.dma_start(out=wv, in_=_rearr(wv_d))
            wo = cp.tile([128, CC, C], F8)
            nc.sync.dma_start(out=wo, in_=_rearr(wo_d))
            pk = cp.tile([128, NT, KLR], F8)
            nc.sync.dma_start(out=pk, in_=_rearr(pk_d))
            pv = cp.tile([128, NT, KLR], F8)
            nc.sync.dma_start(out=pv, in_=_rearr(pv_d))
            ident = cp.tile([128, 128], F8)
            nc.sync.dma_start(out=ident, in_=ident_d[:])
            gcol = cp.tile([128, CC], F32)
            nc.sync.dma_start(out=gcol, in_=gcol_d[:])
            lnb16 = cp.tile([128, CC], F32)
            nc.sync.dma_start(out=lnb16, in_=lnb16_d[:])

            onesb = cp.tile([128, 1], BF16)
            nc.vector.memset(onesb, 1.0)
            onesrow = cp.tile([1, 128], BF16)
            nc.vector.memset(onesrow, 1.0)
            sel8 = cp.tile([8, CC, 128], BF16)
            nc.sync.dma_start(out=sel8, in_=sel8_d[:])
            # sums lhsT: e2b[:, pr, hp, kc, j] = 1 iff j == 2*pr+hp
            # (zero-padded to 128 cols; DoubleRow needs full-width weights)
            e2b = cp.tile([128, CC, 2, 2, 128], F8)
            nc.sync.dma_start(out=e2b, in_=e2b_d[:])
            epsc = cp.tile([1, 1], F32)
            nc.vector.memset(epsc, EPS)

            c = dict(wq=wq, wk=wk, wv=wv, wo=wo, pk=pk, pv=pv, ident=ident,
                     gcol=gcol, lnb16=lnb16, onesb=onesb, onesrow=onesrow,
                     sel8=sel8, e2b=e2b, epsc=epsc, posT=posT)
            with nc.allow_low_precision(reason="fp8 attention path"):
                for _rep in range(reps):
                    fronts = []
                    with (tc.tile_pool(name=f"psA{_rep}", bufs=2,
                                       space="PSUM") as ppa,
                          tc.tile_pool(name=f"wA{_rep}", bufs=2) as wpa):
                        for b0 in range(0, BL, 2):
                            fronts.extend(
                                _emit_front_pair(nc, wp, wpa, ppa,
                                                 (b0, b0 + 1), x_d, c))
                    with (tc.tile_pool(name=f"psB{_rep}", bufs=2,
                                       space="PSUM") as ppb,
                          tc.tile_pool(name=f"wB{_rep}", bufs=2) as wpb):
                        _emit_back_all(nc, wp, wpb, ppb, out_d, c, fronts)
    nc.compile()
    return nc


def _emit_front_pair(nc, wp, wpa, pp, pair, x_d, c):
    """LN + q/k/v projections for a pair of batches, stage-interleaved.
    ACT funcs: Sqrt, Relu, Copy."""
    wq, wk, wv = c["wq"], c["wk"], c["wv"]
    pk, pv = c["pk"], c["pv"]
    gcol, lnb16, epsc = c["gcol"], c["lnb16"], c["epsc"]
    onesb, onesrow, posT = c["onesb"], c["onesrow"], c["posT"]
    F = {b: {} for b in pair}

    # ------- s = x + posT' in bf16 (posT' carries Wo bias) -------
    for b in pair:
        x = wpa.tile([128, CC, N], F32, tag="x", bufs=2)
        nc.sync.dma_start(out=x,
                          in_=x_d[b].rearrange("(a p) n -> p a n", p=128))
        s = wp.tile([128, CC, N], BF16, tag="s", bufs=BL,
                    name=f"s_{b}")
        nc.vector.tensor_add(s, x, posT)
        F[b]["s"] = s

    # ------- LN statistics (sum via s directly; sumsq via pool squares) ----
    for b in pair:
        s = F[b]["s"]
        minis = wpa.tile([1, 2, N], BF16, tag="minis", bufs=2,
                         name=f"minis_{b}")
        m1c = wpa.tile([1, 2, 512], F32, tag="m1c", bufs=2,
                       name=f"m1c_{b}")  # C*mean^2 per half
        for nh in range(NH):
            nsl = slice(nh * 512, (nh + 1) * 512)
            s1 = pp.tile([1, 512], F32, tag="st", bufs=2)
            for cc in range(CC):
                nc.tensor.matmul(s1, onesb[:], s[:, cc, nsl],
                                 start=(cc == 0), stop=(cc == CC - 1))
            mean = minis[0:1, 0, nsl]
            nc.vector.tensor_scalar_mul(mean, s1, 1.0 / C)
            nc.vector.tensor_mul(m1c[0:1, nh, :], mean, s1)  # = C*mean^2
        F[b]["minis"], F[b]["m1c"] = minis, m1c
    for b in pair:
        s = F[b]["s"]
        s2s = [pp.tile([1, 512], F32, tag="st", bufs=2, name=f"s2_{b}_{i}")
               for i in range(NH)]
        for cc in range(CC):
            sq = wpa.tile([128, N], BF16, tag="sq", bufs=4)
            nc.vector.tensor_mul(sq, s[:, cc, :], s[:, cc, :])
            for nh in range(NH):
                nsl = slice(nh * 512, (nh + 1) * 512)
                nc.tensor.matmul(s2s[nh], onesb[:], sq[:, nsl],
                                 start=(cc == 0), stop=(cc == CC - 1))
        F[b]["s2s"] = s2s
    for b in pair:
        minis, m1c = F[b]["minis"], F[b]["m1c"]
        for nh in range(NH):
            nsl = slice(nh * 512, (nh + 1) * 512)
            s2 = F[b]["s2s"][nh]
            v512 = wpa.tile([1, 512], F32, tag="mini2", bufs=4)
            nc.vector.scalar_tensor_tensor(v512, in0=m1c[0:1, nh, :],
                                           scalar=-1.0, in1=s2,
                                           op0=OP.mult, op1=OP.add)  # C*var
            sd = wpa.tile([1, 512], F32, tag="mini2", bufs=4)
            nc.scalar.activation(sd, v512, AF.Sqrt, bias=epsc[:],
                                 scale=1.0 / C)
            nc.vector.reciprocal(minis[0:1, 1, nsl], sd)

    # broadcast mean/rstd down 128 partitions via a replicating SBUF-SBUF DMA
    for b in pair:
        bcs = wpa.tile([128, 2, N], BF16, tag="bcs", bufs=2)
        nc.sync.dma_start(
            out=bcs,
            in_=F[b]["minis"][0:1, None, :, :].broadcast_to([1, 128, 2, N]))
        F[b]["bcs"] = bcs

    # ------- LN apply + relu -> y8 (fp8, x16) -------
    for b in pair:
        F[b]["y8"] = wpa.tile([128, CC, N], F8, tag="y8", bufs=2,
                                  name=f"y8_{b}")
    for cc in range(CC):
        for b in pair:
            s, bcs, y8 = F[b]["s"], F[b]["bcs"], F[b]["y8"]
            t1 = wpa.tile([128, N], BF16, tag="lnt", bufs=4)
            nc.vector.tensor_sub(t1, s[:, cc, :], bcs[:, 0, :])
            t2 = wpa.tile([128, N], BF16, tag="lnt", bufs=4)
            nc.vector.scalar_tensor_tensor(t2, in0=t1,
                                           scalar=gcol[:, cc:cc + 1],
                                           in1=bcs[:, 1, :], op0=OP.mult,
                                           op1=OP.mult)
            nc.scalar.activation(y8[:, cc, :], t2, AF.Relu,
                                 bias=lnb16[:, cc:cc + 1], scale=S_Y)

    # ------- qT [d_part, dc, n] fp8 (x128) -------
    for b in pair:
        F[b]["q8"] = wp.tile([128, CC, N], F8, tag="q8", bufs=BL,
                                 name=f"q8_{b}")
    for dc in range(CC):
        for b in pair:
            y8, q8 = F[b]["y8"], F[b]["q8"]
            ps = pp.tile([128, 2, 512], F32, tag="pj", bufs=3)
            for nh in range(NH):
                nsl = slice(nh * 512, (nh + 1) * 512)
                for kcp in range(0, CC, 2):
                    nc.tensor.matmul(
                        ps[:, nh, :],
                        wq[:, kcp:kcp + 2, dc * 128:(dc + 1) * 128],
                        y8[:, kcp:kcp + 2, nsl], perf_mode=DR,
                        start=(kcp == 0), stop=(kcp == CC - 2),
                        skip_group_check=True)
            psf = ps.rearrange("p a n -> p (a n)")
            if dc % 2 == 0:
                nc.vector.tensor_scalar_mul(q8[:, dc, :], psf,
                                            S_Q / (S_Y * S_WQ))
            else:
                nc.scalar.activation(q8[:, dc, :], psf, AF.Copy,
                                     scale=S_Q / (S_Y * S_WQ))

    # ------- kbf/vbf [n_part, t, d] fp8: transpose fused into the matmul
    # (y8 as the stationary operand), DoubleRow over channel pairs -------
    for which, w8 in (("k", wk), ("v", wv)):
        for b in pair:
            F[b][f"{which}bf"] = wpa.tile([128, NT, C], F8,
                                          tag=f"{which}bf", bufs=2,
                                          name=f"{which}bf_{b}")
        for tp in range(0, NT, 2):
            for b in pair:
                y8 = F[b]["y8"]
                ps = pp.tile([128, 2, 512], F32, tag="pj", bufs=3)
                for i in range(2):
                    t = tp + i
                    for kcp in range(0, CC, 2):
                        nc.tensor.matmul(
                            ps[:, i, :],
                            y8[:, kcp:kcp + 2, t * 128:(t + 1) * 128],
                            w8[:, kcp:kcp + 2, :], perf_mode=DR,
                            start=(kcp == 0), stop=(kcp == CC - 2),
                            skip_group_check=True)
                dst = F[b][f"{which}bf"][:, tp:tp + 2, :]
                psf = ps.rearrange("p a n -> p (a n)")
                if tp % 4 == 0:
                    nc.vector.tensor_scalar_mul(dst, psf, 1.0 / S_W)
                else:
                    nc.scalar.activation(dst, psf, AF.Copy, scale=1.0 / S_W)

    # ------- ktbf [d_part, dc, K] = kbf^T proj_k (x S_K) -------
    for b in pair:
        ktbf = wp.tile([128, CC, KLR], F8, tag="ktbf", bufs=BL,
                       name=f"ktbf_{b}")
        F[b]["ktbf"] = ktbf
    for b in pair:
        kbf = F[b]["kbf"]
        ps = pp.tile([128, 2, 512], F32, tag="pj", bufs=3)
        for dc in range(CC):
            for tp in range(0, NT, 2):
                nc.tensor.matmul(
                    ps[:, dc // 2, (dc % 2) * 256:(dc % 2) * 256 + 256],
                    kbf[:, tp:tp + 2, dc * 128:(dc + 1) * 128],
                    pk[:, tp:tp + 2, :], perf_mode=DR,
                    start=(tp == 0), stop=(tp == NT - 2),
                    skip_group_check=True)
        nc.scalar.activation(F[b]["ktbf"][:, :, :],
                             ps.rearrange("p a n -> p (a n)"), AF.Copy,
                             scale=S_K / (S_Y * S_P))

    # ------- vsbf [k_part, kc, d] = proj_v^T vbf (x S_V) -------
    for b in pair:
        F[b]["vsbf"] = wp.tile([128, KC, C], F8, tag="vsbf", bufs=BL,
                                   name=f"vsbf_{b}")
    for b in pair:
        vbf = F[b]["vbf"]
        ps = pp.tile([128, 2, 512], F32, tag="pj", bufs=3)
        for kc in range(KC):
            for tp in range(0, NT, 2):
                nc.tensor.matmul(ps[:, kc, :],
                                 pv[:, tp:tp + 2, kc * 128:(kc + 1) * 128],
                                 vbf[:, tp:tp + 2, :], perf_mode=DR,
                                 start=(tp == 0), stop=(tp == NT - 2),
                                 skip_group_check=True)
        nc.vector.tensor_scalar_mul(F[b]["vsbf"][:, :, :],
                                    ps.rearrange("p a n -> p (a n)"),
                                    S_V / (S_Y * S_P))

    return [dict(s=F[b]["s"], q8=F[b]["q8"], ktbf=F[b]["ktbf"],
                 vsbf=F[b]["vsbf"]) for b in pair]


def _emit_back_all(nc, wp, wpb, pp, out_d, c, fronts):
    """Attention + Wo + residual. Super-steps over (nh, batch); inside each,
    all 4 head-pairs. Denominators for all 8 heads accumulate into ONE psum
    bank (rows 0:8) -> a single reciprocal per super-step. 3-stage skew keeps
    each engine queue a full stage away from its producers.
    ACT funcs: Exp, Copy."""
    wo, sel8, e2b = c["wo"], c["sel8"], c["e2b"]

    for f in fronts:
        f["aobf"] = wpb.tile([128, CC, N], F8, tag="aobf", bufs=BL,
                             name=f"aobf_{id(f)}")
    steps = [(nh, b) for nh in range(NH) for b in range(len(fronts))]
    state = {}

    def stage_a(i):
        nh, b = steps[i]
        nsl = slice(nh * 512, (nh + 1) * 512)
        f = fronts[b]
        attn = {}
        for pr in range(CC):
            for hp in range(2):
                rsl = slice(hp * 64, (hp + 1) * 64)
                dps = pp.tile([128, 2, 512], F32, tag="dp", bufs=2,
                              name=f"dps_{i}_{pr}_{hp}")
                for kc in range(KC):
                    nc.tensor.matmul(
                        dps[:, kc, :],
                        f["ktbf"][rsl, pr, kc * 128:(kc + 1) * 128],
                        f["q8"][rsl, pr, nsl], start=True, stop=True)
                at = wpb.tile([128, 2, 512], F8, tag="attn", bufs=18,
                              name=f"at_{i}_{pr}_{hp}")
                attn[(pr, hp)] = at
                nc.scalar.activation(at, dps, AF.Exp, scale=1.0 / (S_Q * S_K))
        state[i] = dict(attn=attn)

    def stage_b(i):
        st = state[i]
        sums = pp.tile([128, 512], F32, tag="sm", bufs=2, name=f"sums_{i}")
        for pr in range(CC):
            for hp in range(2):
                nc.tensor.matmul(sums, e2b[:, pr, hp, :, :],
                                 st["attn"][(pr, hp)][:, :, :], perf_mode=DR,
                                 start=(pr == 0 and hp == 0),
                                 stop=(pr == CC - 1 and hp == 1),
                                 skip_group_check=True)
        recip = wpb.tile([8, 512], BF16, tag="recip", bufs=2,
                         name=f"recip_{i}")
        nc.vector.reciprocal(recip, sums[0:8, :])
        st["recip"] = recip

    def stage_c(i):
        nh, b = steps[i]
        nsl = slice(nh * 512, (nh + 1) * 512)
        f = fronts[b]
        st = state.pop(i)
        attn, vsbf = st["attn"], f["vsbf"]
        for pr in range(CC):
            rbc = wpb.tile([128, 512], BF16, tag="rbc", bufs=4,
                           name=f"rbcs_{i}_{pr}")
            nc.sync.dma_start(
                out=rbc[0:64, :],
                in_=st["recip"][2 * pr:2 * pr + 1, None, :]
                .broadcast_to([1, 64, 512]))
            nc.sync.dma_start(
                out=rbc[64:128, :],
                in_=st["recip"][2 * pr + 1:2 * pr + 2, None, :]
                .broadcast_to([1, 64, 512]))
            av = pp.tile([128, 512], F32, tag="rv", bufs=2,
                         name=f"av_{i}_{pr}")
            # hp0 via DoubleRow at rows 0:64; DR cannot write at partition
            # offset 64, so hp1 uses plain fp8 there
            nc.tensor.matmul(av[0:64, :],
                             vsbf[:, :, (2 * pr) * 64:(2 * pr + 1) * 64],
                             attn[(pr, 0)][:, :, :], perf_mode=DR,
                             start=True, stop=True, skip_group_check=True)
            h1 = 2 * pr + 1
            for kc in range(KC):
                nc.tensor.matmul(av[64:128, :],
                                 vsbf[:, kc, h1 * 64:(h1 + 1) * 64],
                                 attn[(pr, 1)][:, kc, :],
                                 start=(kc == 0), stop=(kc == KC - 1),
                                 tile_position=(0, 64), skip_group_check=True)
            nc.vector.scalar_tensor_tensor(f["aobf"][:, pr, nsl], in0=av,
                                           scalar=0.0, in1=rbc,
                                           op0=OP.bypass, op1=OP.mult)

    n = len(steps)
    for i in range(n + 2):
        if i < n:
            stage_a(i)
        if i >= 1 and i - 1 < n:
            stage_b(i - 1)
        if i >= 2:
            stage_c(i - 2)

    # ------- Wo + residual (+bias via posT') -> out -------
    for co in range(CC):
        for b, f in enumerate(fronts):
            outf = wpb.tile([128, N], F32, tag="outf", bufs=4)
            ps = pp.tile([128, 2, 512], F32, tag="dp", bufs=2,
                         name=f"wo_{b}_{co}")
            for nh in range(NH):
                nsl = slice(nh * 512, (nh + 1) * 512)
                for pp_ in range(0, CC, 2):
                    nc.tensor.matmul(ps[:, nh, :],
                                     wo[:, pp_:pp_ + 2,
                                        co * 128:(co + 1) * 128],
                                     f["aobf"][:, pp_:pp_ + 2, nsl],
                                     perf_mode=DR,
                                     start=(pp_ == 0), stop=(pp_ == CC - 2),
                                     skip_group_check=True)
            nc.vector.scalar_tensor_tensor(outf, in0=ps.rearrange(
                                               "p a n -> p (a n)"),
                                           scalar=1.0 / (S_V * S_W),
                                           in1=f["s"][:, co, :],
                                           op0=OP.mult, op1=OP.add)
            nc.sync.dma_start(out=out_d[b, co * 128:(co + 1) * 128, :],
                              in_=outf)


_CACHE = {}


def get_nc(reps=1):
    key = ("nc", reps)
    if key not in _CACHE:
        _CACHE[key] = _build(reps)
    return _CACHE[key]


def _sel8_host():
    sel8 = np.zeros((8, CC, 128), ml_dtypes.bfloat16)
    for pr in range(CC):
        sel8[2 * pr, pr, 0:64] = 1
        sel8[2 * pr + 1, pr, 64:128] = 1
    return sel8


def _e2b_host():
    e2b = np.zeros((128, CC, 2, 2, 128), ml_dtypes.float8_e4m3)
    for pr in range(CC):
        for hp in range(2):
            e2b[:, pr, hp, :, 2 * pr + hp] = 1
    return e2b


def make_in_maps(inputs):
    f8 = ml_dtypes.float8_e4m3
    x = np.ascontiguousarray(np.asarray(inputs["x"], np.float32)
                             .reshape(B, C, N))
    pos = np.asarray(inputs["pos"], np.float32).reshape(N, C)
    ln_g = np.asarray(inputs["ln_g"], np.float32)
    ln_b = np.asarray(inputs["ln_b"], np.float32)
    bo = np.asarray(inputs["bo"], np.float32)

    posT = np.ascontiguousarray(pos.T) + bo[:, None]  # fold Wo bias into pos

    shared = {
        "posT": posT,
        "wq": (np.asarray(inputs["Wq"], np.float32)
               * (DH ** -0.5) * S_WQ).astype(f8),
        "wk": (np.asarray(inputs["Wk"], np.float32) * S_W).astype(f8),
        "wv": (np.asarray(inputs["Wv"], np.float32) * S_W).astype(f8),
        "wo": (np.asarray(inputs["Wo"], np.float32) * S_W).astype(f8),
        "pk": (np.asarray(inputs["proj_k"], np.float32) * S_P).astype(f8),
        "pv": (np.asarray(inputs["proj_v"], np.float32) * S_P).astype(f8),
        "ident": np.eye(128, dtype=f8),
        "sel8": _sel8_host(),
        "e2b": _e2b_host(),
        "gcol": np.ascontiguousarray(ln_g.reshape(CC, 128).T),
        "lnb16": np.ascontiguousarray((S_Y * ln_b).reshape(CC, 128).T),
    }
    return [dict(shared, x=np.ascontiguousarray(x[i * BL:(i + 1) * BL]))
            for i in range(NCORES)]


def kernel(**inputs):
    nc = get_nc()
    in_maps = make_in_maps(inputs)
    trace = bool(int(os.environ.get("BASS_KERNEL_TRACE", "0")))
    res = run_bass_kernel_spmd(nc, in_maps, core_ids=list(range(NCORES)),
                               trace=trace)
    kernel.last_result = res
    out = np.concatenate([np.asarray(res.results[i]["out"], np.float32)
                          [None] for i in range(NCORES)], axis=0)
    return np.ascontiguousarray(out.reshape(B, C, HH, WW))
